# revision 6
# baseline (speedup 1.0000x reference)
"""BNB 8-bit embedding lookup (gather + dequant) on 8 Trainium2 NeuronCores.

out[b, s, :] = q_weight[x[b, s]].astype(f32) * (absmax[x[b, s]] / 127)

Sharding: pure data-parallel over tokens. x is [8, 4096] and there are 8
cores, so core c handles batch row c (4096 tokens) and produces out[c]
with no cross-core communication. The quantized table is replicated on
every core, packed host-side into rows of
[1024B int8 payload | 4B f32 scale (= absmax/127)] so a single
indirect-DMA descriptor per token fetches the weights and their dequant
scale together.

Key perf choices vs the first (f32-out) version of this kernel:
  * The device writes the output in fp16 (half the HBM store traffic;
    the dominant cost at this shape) and the host upcasts to f32. fp16
    round-off is <= 2^-11 relative, far inside the 2e-2 gate.
  * Token layout is p-major: partition p owns tokens p*32 .. p*32+31.
    Each 8-column store then writes 16 KB contiguous per partition
    (near line-rate HBM writes) instead of 2 KB strided chunks.
  * Gathers are per-column ([128, 1] offsets): HW honors only the first
    offset column per indirect DMA (multi-column offsets verified broken
    on silicon — only 1/G of the data lands), so 32 gather instructions
    per rep it is, issued alternating between the scalar-engine half and
    the vector-engine half so both dequant engines start early.
  * Dequant is split across the Vector and Scalar engines (16 columns
    each). int8 input forces DVE 1x mode (~34us for all 32 tiles on one
    engine, which would exceed the ~35us HBM floor); split, each engine
    does ~17us and stays hidden under the DMA traffic.

Per-core HBM traffic: 4.2MB gather reads + 8.4MB fp16 writes = 12.6MB
against ~358GB/s per-NC HBM bandwidth -> ~35us floor.

If q_weight arrives in a wider integer range than int8 (e.g. a harness
that fills uint8-range values in an int32 array), the same kernel is
built with an int16 payload (2052B rows) instead.
"""

import numpy as np

from concourse import bass, bacc, mybir, tile
from concourse import bass_utils

VOCAB = 50257
DIM = 1024
B, S = 8, 4096
N_CORES = 8
P = 128
TOK_PER_CORE = S             # core c <- batch row c
COLS = TOK_PER_CORE // P     # 32 tokens per partition, p-major layout
GRP = 8                      # columns per gather / per store
N_GRPS = COLS // GRP         # 4

# payload dtype -> (row bytes incl. 4B scale, payload bytes)
_LAYOUTS = {
    "int8": (1028, DIM),       # 1024 q + 4 scale f32
    "int16": (2052, 2 * DIM),  # 2048 q + 4 scale f32
}

GBUFS = 12        # gather-tile pool slots (1028B/partition each)
OBUFS = 6         # output-tile pool slots (16KB/partition each)

_PROGRAMS: dict = {}


def _build_program(payload: str, reps: int = 1):
    # reps > 1 repeats the whole body inside one NEFF; used only by the
    # local perf harness (test.py) to difference out dispatch overhead.
    row_bytes, q_bytes = _LAYOUTS[payload]

    nc = bacc.Bacc("TRN2", target_bir_lowering=False, debug=False,
                   num_devices=N_CORES)
    xt = nc.dram_tensor("xt", [P, COLS], mybir.dt.int32,
                        kind="ExternalInput").ap()
    table = nc.dram_tensor("table", [VOCAB, row_bytes], mybir.dt.int8,
                           kind="ExternalInput").ap()
    out = nc.dram_tensor("out", [TOK_PER_CORE, DIM], mybir.dt.float16,
                         kind="ExternalOutput").ap()
    out_r = out.rearrange("(p j) d -> p j d", p=P)

    with tile.TileContext(nc) as tc:
        with tc.tile_pool(name="idx", bufs=2) as idx_pool, \
             tc.tile_pool(name="g", bufs=GBUFS) as gpool, \
             tc.tile_pool(name="o", bufs=OBUFS) as opool:
            for _rep in range(reps):
                x_sb = idx_pool.tile([P, COLS], mybir.dt.int32)
                nc.sync.dma_start(out=x_sb[:], in_=xt[:])
                # Columns 0..15 dequant on ACT, 16..31 on DVE. Gather in
                # alternating order 0,16,1,17,... so both engines get
                # work as early as possible.
                half = COLS // 2
                gts: dict = {}
                for k in range(half):
                    for j in (k, half + k):
                        gt = gpool.tile([P, row_bytes], mybir.dt.int8,
                                        tag="g")
                        nc.gpsimd.indirect_dma_start(
                            out=gt[:], out_offset=None,
                            in_=table[:],
                            in_offset=bass.IndirectOffsetOnAxis(
                                ap=x_sb[:, j:j + 1], axis=0),
                        )
                        gts[j] = gt
                os: dict = {}
                for g in range(N_GRPS):
                    os[g] = opool.tile([P, GRP, DIM], mybir.dt.float16,
                                       tag="o", name=f"o{g}")
                for k in range(half):
                    for j in (k, half + k):
                        gt = gts[j]
                        o = os[j // GRP]
                        pay = gt[:, 0:q_bytes]
                        if payload == "int16":
                            pay = pay.bitcast(mybir.dt.int16)
                        scale = gt[:, q_bytes:q_bytes + 4]\
                            .bitcast(mybir.dt.float32)
                        if j < half:
                            nc.scalar.mul(o[:, j % GRP, :], pay[:, 0:DIM],
                                          scale)
                        else:
                            nc.vector.tensor_scalar_mul(
                                out=o[:, j % GRP, :], in0=pay[:, 0:DIM],
                                scalar1=scale)
                for g in range(N_GRPS):
                    dst = out_r[:, g * GRP:(g + 1) * GRP, :]
                    eng = nc.sync if g % 2 == 0 else nc.scalar
                    eng.dma_start(out=dst, in_=os[g][:])

    nc.compile()
    return nc


def _get_program(payload: str, reps: int = 1):
    key = (payload, reps)
    if key not in _PROGRAMS:
        _PROGRAMS[key] = _build_program(payload, reps)
    return _PROGRAMS[key]


def _pack_table(q_weight: np.ndarray, absmax: np.ndarray, payload: str):
    row_bytes, q_bytes = _LAYOUTS[payload]
    np_dt = np.int8 if payload == "int8" else np.int16
    packed = np.zeros((VOCAB, row_bytes), dtype=np.int8)
    packed[:, :q_bytes] = q_weight.astype(np_dt, copy=False).view(np.int8)
    scales = (absmax.astype(np.float32, copy=False)
              * np.float32(1.0 / 127.0)).reshape(-1, 1)
    packed[:, q_bytes:q_bytes + 4] = scales.view(np.int8)
    return packed


def _make_xt(x_row):
    # p-major: partition p owns tokens p*COLS .. p*COLS+COLS-1
    x_row = np.ascontiguousarray(x_row).astype(np.int32, copy=False)
    return np.ascontiguousarray(x_row.reshape(P, COLS))


def kernel(x=None, q_weight=None, absmax=None, **_ignored):
    x = np.asarray(x)
    q_weight = np.asarray(q_weight)
    absmax = np.asarray(absmax)
    assert x.shape == (B, S), x.shape
    assert q_weight.shape == (VOCAB, DIM), q_weight.shape

    qmin, qmax = int(q_weight.min()), int(q_weight.max())
    payload = "int8" if (-128 <= qmin and qmax <= 127) else "int16"

    nc = _get_program(payload)
    packed = _pack_table(q_weight, absmax, payload)

    x_i32 = x.astype(np.int32, copy=False)
    in_maps = [{"xt": _make_xt(x_i32[c]), "table": packed}
               for c in range(N_CORES)]

    res = bass_utils.run_bass_kernel_spmd(
        nc, in_maps, core_ids=list(range(N_CORES)))
    out16 = np.stack([res.results[c]["out"] for c in range(N_CORES)], axis=0)
    return out16.astype(np.float32)
